# revision 1
# baseline (speedup 1.0000x reference)
"""Trainium2 Bass kernel v2.1 for the GCN model (8 NeuronCores).

v2.1 on top of v2:
- No layer-boundary fix loop: the epilogue writes shard rows v = dinv*(h'@W)
  directly (pre-LayerNorm).  The LN affine moves into the NEXT layer's
  epilogue using the host-precomputed neighbor weight sum
  sigma_d = dinv_d + sum_{s in N(d)} dinv_s:
     true_pre_relu = rstd_prev * (dinv*aggP - mu_prev * wbar (x) sigma*dinv) + b
  applied via the relu activation's per-partition scale (rstd) plus one
  rank-1 correction built from a replicated sigma*dinv row and a per-feature
  wbar*mu column.  Shard writes and both AllGather halves therefore ride
  inside the window-3 gather stream; the stats AllReduce latency hides under
  the next layer's window-0 gather.
- All psum->SBUF copies (self-init, P stash, h3, pool) moved to the Scalar
  engine (activation Copy, with the dinv scaling folded into `scale`);
  relu writes bf16 h' directly.  VectorE keeps only seg builds, flushes,
  the dinv/corr muls and stats adds.
- xT/lin1W in bf16, P0 processed from 7 bulk slabs.
"""

import sys

sys.path.insert(0, "/opt/trn_rl_repo")

import numpy as np
import ml_dtypes

import concourse.bass as bass
import concourse.bacc as bacc
import concourse.mybir as mybir
import concourse.tile as tile
from concourse.bass_utils import run_bass_kernel_spmd
from concourse.library_config import mlp as gpsimd_mlp_lib
from concourse.masks import make_identity

NCORES = 8
N_NODES = 100_000
F = 128
NCLS = 10
LAYERS = 3
NGRAPH = 256
EPS = 1e-5
NSH = N_NODES // NCORES           # 12500
NBLK = 98
LASTW = NSH - (NBLK - 1) * 128    # 84
A_BLKS = 49
A_ROWS = A_BLKS * 128             # 6272
B_ROWS = NSH - A_ROWS             # 6228
REG_A = NCORES * A_ROWS           # 50176
REG_B = NCORES * B_ROWS           # 49824
WIN_BASE = [0, 4 * A_ROWS, REG_A, REG_A + 4 * B_ROWS]
WIN_SIZE = [4 * A_ROWS, 4 * A_ROWS, 4 * B_ROWS, 4 * B_ROWS]
NWIN = 4
CHUNK = 1024
TPC = CHUNK // 128
SLABCH = 16
NQUEUE = 4
GBUF = 14
P0SLAB = 14                        # blocks per P0 x-slab (7 slabs)

BF16 = mybir.dt.bfloat16
F32 = mybir.dt.float32
I16 = mybir.dt.int16
AOP = mybir.AluOpType
AF = mybir.ActivationFunctionType
BF = ml_dtypes.bfloat16


def _host_preprocess(edge_index):
    """Window/tile layout + gather indices.  Self-loops NOT added here."""
    src = np.asarray(edge_index[0], dtype=np.int64)
    dst = np.asarray(edge_index[1], dtype=np.int64)
    deg = np.bincount(dst, minlength=N_NODES).astype(np.float64) + 1.0
    dinv = (1.0 / np.sqrt(deg)).astype(np.float32)
    sig = dinv.astype(np.float64) + np.bincount(
        dst, weights=dinv[src].astype(np.float64), minlength=N_NODES)
    sigdinv = (sig * dinv).astype(np.float32)

    # source -> (window, in-window idx) under the permuted hws_full layout
    c_s = src // NSH
    r = src % NSH
    half = (r >= A_ROWS).astype(np.int64)
    win = half * 2 + (c_s // 4)
    grow = np.where(half == 0,
                    c_s * A_ROWS + r,
                    REG_A + c_s * B_ROWS + (r - A_ROWS))
    widx = grow - np.asarray(WIN_BASE, dtype=np.int64)[win]

    core = dst // NSH
    blk = (dst % NSH) // 128
    slot = (dst % NSH) % 128

    key = (core * NWIN + win) * NBLK + blk
    order = np.argsort(key, kind="stable")
    key_s = key[order]
    widx_s = widx[order]
    slot_s = slot[order]
    ngroups = NCORES * NWIN * NBLK
    counts = np.bincount(key_s, minlength=ngroups).reshape(NCORES, NWIN, NBLK)
    starts = np.zeros(ngroups + 1, dtype=np.int64)
    np.cumsum(counts.reshape(-1), out=starts[1:])

    H = (counts.max(axis=0) + 63) // 64            # [NWIN, NBLK] halves
    padH = np.zeros(NWIN, dtype=np.int64)
    for w in range(NWIN):
        padH[w] = (-int(H[w].sum())) % (TPC * 2)
    NHALF = int(H.sum() + padH.sum())
    TT = NHALF // 2
    ecap = NHALF * 64
    nchunk = ecap // CHUNK

    half_blk = np.empty(NHALF, dtype=np.int64)
    half_win = np.empty(NHALF, dtype=np.int64)
    gfirst = np.full((NWIN, NBLK), -1, dtype=np.int64)
    glast = np.full((NWIN, NBLK), -1, dtype=np.int64)
    hstart = np.zeros((NWIN, NBLK), dtype=np.int64)
    t0 = 0
    for w in range(NWIN):
        lastb = -1
        for b in range(NBLK):
            n = int(H[w, b])
            hstart[w, b] = t0
            if n == 0:
                continue
            half_blk[t0:t0 + n] = b
            half_win[t0:t0 + n] = w
            gfirst[w, b] = t0
            glast[w, b] = t0 + n - 1
            t0 += n
            lastb = b
        p = int(padH[w])
        if p:
            half_blk[t0:t0 + p] = lastb
            half_win[t0:t0 + p] = w
            glast[w, lastb] = t0 + p - 1
            t0 += p
    assert t0 == NHALF
    last_win = np.full(NBLK, -1, dtype=np.int64)
    for w in range(NWIN):
        for b in range(NBLK):
            if H[w, b] > 0:
                last_win[b] = max(last_win[b], w)
    assert (last_win >= 0).all()

    idx16 = np.zeros((NCORES, ecap), dtype=np.int16)
    slots = np.full((NCORES, ecap), 300, dtype=np.float32)
    for c in range(NCORES):
        for w in range(NWIN):
            for b in range(NBLK):
                if H[w, b] == 0:
                    continue
                g = (c * NWIN + w) * NBLK + b
                s0, s1 = starts[g], starts[g + 1]
                n = s1 - s0
                pos = hstart[w, b] * 64
                idx16[c, pos:pos + n] = widx_s[s0:s1].astype(np.int16)
                slots[c, pos:pos + n] = slot_s[s0:s1].astype(np.float32)
    # +128 tags for odd halves that start a new block inside a tile
    for h in range(1, NHALF, 2):
        if half_blk[h] != half_blk[h - 1]:
            sl = slots[:, h * 64:(h + 1) * 64]
            sl[sl < 256] += 128.0

    idxw = idx16.reshape(NCORES, -1, 16).transpose(0, 2, 1)
    idxw = np.ascontiguousarray(np.tile(idxw, (1, 8, 1)))
    slotw = np.ascontiguousarray(
        slots.reshape(NCORES, TT, 128).transpose(0, 2, 1)).astype(BF)

    meta = dict(TT=TT, nchunk=nchunk, half_blk=half_blk, half_win=half_win,
                gfirst=gfirst, glast=glast, last_win=last_win)
    return dinv, sigdinv, idxw, slotw, meta


def _build_program(meta):
    import os
    TT = meta["TT"]
    NCHUNK = meta["nchunk"]
    half_blk = meta["half_blk"]
    half_win = meta["half_win"]
    gfirst = meta["gfirst"]
    glast = meta["glast"]
    last_win = meta["last_win"]
    ECAP = TT * 128
    cut = os.environ.get("KCUT", "full")
    nlayers_run = {"p0": 0, "l0": 1, "l01": 2}.get(cut, LAYERS)

    nc = bacc.Bacc("TRN2", target_bir_lowering=False, debug=False,
                   num_devices=NCORES, num_swdge_queues=NQUEUE)

    xT_in = nc.declare_dram_parameter("xT", [F, NBLK * 128], BF16, isOutput=False)
    idx_in = nc.declare_dram_parameter("idx", [128, ECAP // 16], I16, isOutput=False)
    slot_in = nc.declare_dram_parameter("slot", [128, TT], BF16, isOutput=False)
    dinvrep_in = nc.declare_dram_parameter("dinvrep", [128, NBLK * 128], BF16, isOutput=False)
    sigdr_in = nc.declare_dram_parameter("sigdr", [128, NBLK * 128], BF16, isOutput=False)
    dinvw_in = nc.declare_dram_parameter("dinvw", [128, NBLK], F32, isOutput=False)
    pslot_in = nc.declare_dram_parameter("pslot", [128, NBLK], BF16, isOutput=False)
    iota8_in = nc.declare_dram_parameter("iota8", [128, CHUNK], BF16, isOutput=False)
    iota256_in = nc.declare_dram_parameter("iota256", [128, 256], BF16, isOutput=False)
    lin1W_in = nc.declare_dram_parameter("lin1W", [F, F], BF16, isOutput=False)
    lin1b_in = nc.declare_dram_parameter("lin1b", [F, 1], F32, isOutput=False)
    convW_in = nc.declare_dram_parameter("convW", [F, LAYERS * F], BF16, isOutput=False)
    convb_in = nc.declare_dram_parameter("convb", [F, LAYERS], F32, isOutput=False)
    wbarT_in = nc.declare_dram_parameter("wbarT", [F, LAYERS - 1], F32, isOutput=False)
    mlpW1_in = nc.declare_dram_parameter("mlpW1", [F, F], BF16, isOutput=False)
    mlpb1_in = nc.declare_dram_parameter("mlpb1", [F, 1], F32, isOutput=False)
    mlpW2_in = nc.declare_dram_parameter("mlpW2", [F, NCLS], BF16, isOutput=False)
    mlpb2r_in = nc.declare_dram_parameter("mlpb2r", [128, NCLS], F32, isOutput=False)
    invcntr_in = nc.declare_dram_parameter("invcntr", [128, NGRAPH], F32, isOutput=False)
    out_ext = nc.declare_dram_parameter("out", [NGRAPH, NCLS], F32, isOutput=True)

    rg = [list(range(NCORES))]

    with tile.TileContext(nc) as tc:
        with tc.tile_pool(name="const", bufs=1) as cst, \
             tc.tile_pool(name="big", bufs=1) as big, \
             tc.tile_pool(name="work", bufs=3) as work, \
             tc.tile_pool(name="segp", bufs=4) as segpool, \
             tc.tile_pool(name="gbuf", bufs=GBUF) as gpool, \
             tc.tile_pool(name="idxs", bufs=3) as ipool, \
             tc.tile_pool(name="psum", bufs=4, space="PSUM") as pp, \
             tc.tile_pool(name="mmp", bufs=3, space="PSUM") as mmp, \
             tc.tile_pool(name="ppool", bufs=1, space="PSUM") as ppool, \
             tc.tile_pool(name="dram", bufs=1, space="DRAM") as dram, \
             tc.tile_pool(name="dram2", bufs=1, space="DRAM") as dram2:

            nc.gpsimd.load_library(gpsimd_mlp_lib)

            slot_t = cst.tile([128, TT], BF16)
            nc.sync.dma_start(out=slot_t[:], in_=slot_in[:])
            dinvrep = cst.tile([128, NBLK * 128], BF16)
            nc.sync.dma_start(out=dinvrep[:], in_=dinvrep_in[:])
            sigdr = cst.tile([128, NBLK * 128], BF16)
            nc.sync.dma_start(out=sigdr[:], in_=sigdr_in[:])
            dinvw = cst.tile([128, NBLK], F32)
            nc.sync.dma_start(out=dinvw[:], in_=dinvw_in[:])
            pslot = cst.tile([128, NBLK], BF16)
            nc.sync.dma_start(out=pslot[:], in_=pslot_in[:])
            iota8 = cst.tile([128, CHUNK], BF16)
            nc.sync.dma_start(out=iota8[:], in_=iota8_in[:])
            iota256 = cst.tile([128, 256], BF16)
            nc.sync.dma_start(out=iota256[:], in_=iota256_in[:])
            lin1W = cst.tile([F, F], BF16)
            nc.sync.dma_start(out=lin1W[:], in_=lin1W_in[:])
            lin1b = cst.tile([F, 1], F32)
            nc.sync.dma_start(out=lin1b[:], in_=lin1b_in[:])
            convW = cst.tile([F, LAYERS * F], BF16)
            nc.sync.dma_start(out=convW[:], in_=convW_in[:])
            convb = cst.tile([F, LAYERS], F32)
            nc.sync.dma_start(out=convb[:], in_=convb_in[:])
            wbarT = cst.tile([F, LAYERS - 1], F32)
            nc.sync.dma_start(out=wbarT[:], in_=wbarT_in[:])
            invcntr = cst.tile([128, NGRAPH], F32)
            nc.sync.dma_start(out=invcntr[:], in_=invcntr_in[:])
            mlpW1 = cst.tile([F, F], BF16)
            nc.sync.dma_start(out=mlpW1[:], in_=mlpW1_in[:])
            mlpb1 = cst.tile([F, 1], F32)
            nc.sync.dma_start(out=mlpb1[:], in_=mlpb1_in[:])
            mlpW2 = cst.tile([F, NCLS], BF16)
            nc.sync.dma_start(out=mlpW2[:], in_=mlpW2_in[:])
            mlpb2r = cst.tile([128, NCLS], F32)
            nc.sync.dma_start(out=mlpb2r[:], in_=mlpb2r_in[:])
            ones_col = cst.tile([128, 1], BF16)
            nc.vector.memset(ones_col[:], 1.0)
            ones_row1 = cst.tile([1, 128], BF16)
            nc.vector.memset(ones_row1[:], 1.0)
            ident = cst.tile([128, 128], BF16)
            make_identity(nc, ident[:])

            acc = big.tile([128, NBLK * 128], F32)
            stash = big.tile([128, NBLK * 128], BF16)
            nc.vector.memset(stash[:], 0.0)

            for _ in range(GBUF):
                g0 = gpool.tile([128, TPC, F], BF16, tag="gb")
                nc.vector.memset(g0[:].rearrange("p a b -> p (a b)"), 0.0)

            hwsA = dram.tile([A_ROWS, F], BF16)
            hwsB = dram.tile([B_ROWS, F], BF16)
            hwsfA = [dram.tile([REG_A, F], BF16, addr_space="Shared",
                                name=f"hwsfA{l}") for l in range(LAYERS)]
            hwsfB = [dram.tile([REG_B, F], BF16, addr_space="Shared",
                               name=f"hwsfB{l}") for l in range(LAYERS)]

            def win_ap_l(li, w):
                if w < 2:
                    return hwsfA[li][w * (4 * A_ROWS):(w + 1) * (4 * A_ROWS), :]
                return hwsfB[li][(w - 2) * (4 * B_ROWS):
                                 (w - 1) * (4 * B_ROWS), :]

            def bw(b):
                return 128 if b < NBLK - 1 else LASTW

            def shard_write(b):
                w = bw(b)
                src = stash[:w, b * 128:(b + 1) * 128]
                if b < A_BLKS:
                    nc.sync.dma_start(out=hwsA[b * 128:b * 128 + w, :], in_=src)
                else:
                    o = (b - A_BLKS) * 128
                    nc.sync.dma_start(out=hwsB[o:o + w, :], in_=src)

            def emit_selfinit(b):
                ps_t = mmp.tile([128, 128], BF16, tag="mm")
                nc.tensor.transpose(out=ps_t[:],
                                    in_=stash[:, b * 128:(b + 1) * 128],
                                    identity=ident[:])
                nc.scalar.activation(out=acc[:, b * 128:(b + 1) * 128],
                                     in_=ps_t[:], func=AF.Copy,
                                     bias=0.0, scale=1.0)

            def emit_ag(which, idx):
                if which == 0:
                    nc.gpsimd.collective_compute(
                        "AllGather", AOP.bypass, replica_groups=rg,
                        ins=[hwsA[:]], outs=[hwsfA[idx][:]])
                else:
                    nc.gpsimd.collective_compute(
                        "AllGather", AOP.bypass, replica_groups=rg,
                        ins=[hwsB[:]], outs=[hwsfB[idx][:]])

            # ---------------- P0 (B-half first so AG-B hides here) ----------
            pending_ag = [False]
            p0_ranges = []
            for lo, hi in ((0, A_BLKS), (A_BLKS, NBLK)):
                s = lo
                while s < hi:
                    p0_ranges.append((s, min(P0SLAB, hi - s)))
                    s += min(P0SLAB, hi - s)
            for (s0, ns) in p0_ranges:
                xsl = work.tile([128, P0SLAB * 128], BF16, tag="xsl")
                nc.sync.dma_start(
                    out=xsl[:, :ns * 128],
                    in_=xT_in[:, s0 * 128:(s0 + ns) * 128])
                for j in range(ns):
                    b = s0 + j
                    w = bw(b)
                    ps = mmp.tile([128, 128], F32, tag="mm")
                    nc.tensor.matmul(out=ps[:], lhsT=lin1W[:],
                                     rhs=xsl[:, j * 128:(j + 1) * 128],
                                     start=True, stop=True)
                    h1b = work.tile([128, 128], BF16, tag="h1")
                    nc.scalar.activation(out=h1b[:], in_=ps[:], func=AF.Relu,
                                         bias=lin1b[:], scale=1.0)
                    ps2 = mmp.tile([128, F], F32, tag="mm")
                    nc.tensor.matmul(out=ps2[:w, :], lhsT=h1b[:, :w],
                                     rhs=convW[:, 0:F], start=True, stop=True)
                    nc.scalar.activation(
                        out=stash[:w, b * 128:(b + 1) * 128], in_=ps2[:w, :],
                        func=AF.Copy, bias=0.0, scale=dinvw[:w, b:b + 1])
                    shard_write(b)
                    emit_selfinit(b)
                    if b == A_BLKS - 1:
                        emit_ag(0, 0)
            pending_ag[0] = True

            # ---------------- conv layers ----------------
            pool_ps = None
            pending_chain = None
            layer_ctx = {"musd": None, "wm": None}
            for li in range(nlayers_run):
                last = li == LAYERS - 1
                stats = work.tile([128, 2], F32, tag="stats")
                nc.vector.memset(stats[:], 0.0)
                if last:
                    pool_ps = ppool.tile([128, NGRAPH], F32, tag="pool")
                nepi = 0

                def emit_epilogue(b, _li=li, _last=last, _stats=stats,
                                  _ctx=layer_ctx):
                    nonlocal nepi, pool_ps
                    w = bw(b)
                    ab = acc[:, b * 128:b * 128 + w]
                    nc.vector.tensor_tensor(
                        out=ab, in0=ab,
                        in1=dinvrep[:, b * 128:b * 128 + w], op=AOP.mult)
                    if _li > 0:
                        corr = work.tile([128, 128], F32, tag="corr")
                        nc.scalar.activation(
                            out=corr[:, :w],
                            in_=sigdr[:, b * 128:b * 128 + w],
                            func=AF.Copy, bias=0.0, scale=_ctx["wm"][:])
                        nc.vector.tensor_tensor(
                            out=ab, in0=ab, in1=corr[:, :w], op=AOP.subtract)
                    hb16 = work.tile([128, 128], BF16, tag="hb16")
                    s1 = work.tile([128, 1], F32, tag="s1")
                    nc.scalar.activation(
                        out=hb16[:, :w], in_=ab, func=AF.Relu,
                        bias=convb[:, _li:_li + 1],
                        scale=(_ctx["musd"][:, 3:4] if _li > 0 else 1.0),
                        accum_out=s1[:])
                    sq = work.tile([128, 128], F32, tag="sq")
                    s2 = work.tile([128, 1], F32, tag="s2")
                    nc.scalar.activation(out=sq[:, :w], in_=hb16[:, :w],
                                         func=AF.Square, bias=0.0, scale=1.0,
                                         accum_out=s2[:])
                    nc.vector.tensor_tensor(out=_stats[:, 0:1], in0=_stats[:, 0:1],
                                            in1=s1[:], op=AOP.add)
                    nc.vector.tensor_tensor(out=_stats[:, 1:2], in0=_stats[:, 1:2],
                                            in1=s2[:], op=AOP.add)
                    if not _last:
                        psP = mmp.tile([128, F], F32, tag="mm")
                        nc.tensor.matmul(
                            out=psP[:w, :], lhsT=hb16[:, :w],
                            rhs=convW[:, (_li + 1) * F:(_li + 2) * F],
                            start=True, stop=True)
                        nc.scalar.activation(
                            out=stash[:w, b * 128:(b + 1) * 128],
                            in_=psP[:w, :], func=AF.Copy, bias=0.0,
                            scale=dinvw[:w, b:b + 1])
                        shard_write(b)
                        emit_selfinit(b)
                        if b == A_BLKS - 1:
                            emit_ag(0, _li + 1)
                        elif b == NBLK - 1:
                            pending_ag[0] = True
                    else:
                        ps_t = mmp.tile([128, 128], BF16, tag="mm")
                        nc.tensor.transpose(out=ps_t[:], in_=hb16[:],
                                            identity=ident[:])
                        h3 = work.tile([128, 128], BF16, tag="h3")
                        nc.scalar.activation(out=h3[:w, :], in_=ps_t[:w, :],
                                             func=AF.Copy, bias=0.0, scale=1.0)
                        segg = work.tile([128, NGRAPH], BF16, tag="segg")
                        nc.vector.tensor_tensor(
                            out=segg[:w, :],
                            in0=pslot[:w, b:b + 1].to_broadcast([w, NGRAPH]),
                            in1=iota256[:w, :], op=AOP.is_equal)
                        nc.tensor.matmul(out=pool_ps[:], lhsT=h3[:w, :],
                                         rhs=segg[:w, :],
                                         start=(nepi == 0),
                                         stop=(nepi == NBLK - 1),
                                         skip_group_check=True)
                    nepi += 1

                # ---- gather + aggregate ----
                gtile = 0
                open_psum = None
                open_blk = -1
                idx_slab = None
                for ch in range(NCHUNK):
                    if ch == 96 and pending_ag[0]:
                        emit_ag(1, li)
                        pending_ag[0] = False
                    if ch == 48 and pending_chain is not None:
                        pending_chain()
                        pending_chain = None
                    if ch % SLABCH == 0:
                        idx_slab = ipool.tile([128, SLABCH * CHUNK // 16], I16,
                                              tag="idxslab")
                        wsl = min(SLABCH * CHUNK, ECAP - ch * CHUNK) // 16
                        eng = nc.sync if ch == 0 else nc.scalar
                        eng.dma_start(
                            out=idx_slab[:, :wsl],
                            in_=idx_in[:, ch * CHUNK // 16:
                                       ch * CHUNK // 16 + wsl])
                    wq = int(half_win[gtile * 2])
                    assert all(int(half_win[gtile * 2 + t]) == wq
                               for t in range(TPC * 2))
                    gb = gpool.tile([128, TPC, F], BF16, tag="gb")
                    off = (ch % SLABCH) * (CHUNK // 16)
                    nc.gpsimd.dma_gather(
                        gb[:], win_ap_l(li, wq),
                        idx_slab[:, off:off + CHUNK // 16],
                        CHUNK, CHUNK, F, single_packet=True,
                        queue_num=ch % NQUEUE)
                    gbf = gb[:].rearrange("p a b -> p (a b)")
                    seg_big = segpool.tile([128, CHUNK], BF16, tag="seg")
                    nc.vector.tensor_tensor(
                        out=seg_big[:].rearrange("p (a b) -> p a b", a=TPC),
                        in0=iota8[:].rearrange("p (a b) -> p a b", a=TPC),
                        in1=slot_t[:, gtile:gtile + TPC].unsqueeze(2)
                            .to_broadcast([128, TPC, 128]),
                        op=AOP.is_equal)

                    def close_blk(b, _wq=None):
                        nonlocal open_psum
                        nc.vector.tensor_tensor(
                            out=acc[:, b * 128:(b + 1) * 128],
                            in0=acc[:, b * 128:(b + 1) * 128],
                            in1=open_psum[:], op=AOP.add)
                        open_psum = None
                        if _wq == int(last_win[b]):
                            emit_epilogue(b)

                    for t in range(TPC):
                        h1 = gtile * 2
                        h2 = h1 + 1
                        b1 = int(half_blk[h1])
                        b2 = int(half_blk[h2])
                        st1 = h1 == int(gfirst[wq][b1])
                        if st1:
                            open_psum = pp.tile([128, 128], F32, tag="agg")
                            open_blk = b1
                        assert open_blk == b1, (gtile, b1, open_blk)
                        if b1 == b2:
                            sp1 = h2 == int(glast[wq][b1])
                            nc.tensor.matmul(
                                out=open_psum[:],
                                lhsT=gbf[:, t * F:(t + 1) * F],
                                rhs=seg_big[:, t * 128:(t + 1) * 128],
                                start=st1, stop=sp1)
                            if sp1:
                                close_blk(b1, wq)
                        else:
                            assert h1 == int(glast[wq][b1])
                            nc.tensor.matmul(
                                out=open_psum[:],
                                lhsT=gbf[:, t * F:(t + 1) * F],
                                rhs=seg_big[:, t * 128:(t + 1) * 128],
                                start=st1, stop=True)
                            close_blk(b1, wq)
                            segB = work.tile([128, 128], BF16, tag="segB")
                            nc.vector.tensor_tensor(
                                out=segB[:],
                                in0=iota256[:, 128:256],
                                in1=slot_t[:, gtile:gtile + 1]
                                    .to_broadcast([128, 128]),
                                op=AOP.is_equal)
                            sp2 = h2 == int(glast[wq][b2])
                            open_psum = pp.tile([128, 128], F32, tag="agg")
                            open_blk = b2
                            nc.tensor.matmul(
                                out=open_psum[:],
                                lhsT=gbf[:, t * F:(t + 1) * F],
                                rhs=segB[:],
                                start=True, stop=sp2)
                            if sp2:
                                close_blk(b2, wq)
                        gtile += 1
                assert gtile == TT

                # ---- stats AllReduce + scalars (deferred into next
                # layer's gather loop so it doesn't head-of-line block) ----
                def emit_stats_chain(_stats=stats, _ctx=layer_ctx, _li=li,
                                     _last=last):
                    st_in = dram2.tile([128, 2], F32, tag="stin")
                    st_out = dram2.tile([128, 2], F32, tag="stout")
                    nc.sync.dma_start(out=st_in[:], in_=_stats[:])
                    nc.gpsimd.collective_compute(
                    "AllReduce", AOP.add, replica_groups=rg,
                    ins=[st_in[:]], outs=[st_out[:]])
                    stg = work.tile([128, 2], F32, tag="stg")
                    nc.sync.dma_start(out=stg[:], in_=st_out[:])
                    stg16 = work.tile([128, 2], BF16, tag="stg16")
                    nc.vector.tensor_copy(out=stg16[:], in_=stg[:])
                    ps_s = mmp.tile([1, 2], F32, tag="mm")
                    nc.tensor.matmul(out=ps_s[:], lhsT=ones_col[:], rhs=stg16[:],
                                 start=True, stop=True)
                    sc = work.tile([1, 4], F32, tag="sc")
                    nc.scalar.activation(out=sc[:, 0:2], in_=ps_s[:], func=AF.Copy,
                                     bias=0.0, scale=1.0 / (N_NODES * F))
                    nc.vector.tensor_tensor(out=sc[:, 2:3], in0=sc[:, 0:1],
                                        in1=sc[:, 0:1], op=AOP.mult)
                    nc.vector.tensor_tensor(out=sc[:, 2:3], in0=sc[:, 1:2],
                                        in1=sc[:, 2:3], op=AOP.subtract)
                    nc.vector.tensor_scalar(out=sc[:, 2:3], in0=sc[:, 2:3],
                                        scalar1=EPS, scalar2=None, op0=AOP.add)
                    nc.vector.reciprocal(out=sc[:, 3:4], in_=sc[:, 2:3])
                    nc.scalar.activation(out=sc[:, 3:4], in_=sc[:, 3:4],
                                     func=AF.Sqrt, bias=0.0, scale=1.0)
                    sc16 = work.tile([1, 4], BF16, tag="sc16")
                    nc.vector.tensor_copy(out=sc16[:], in_=sc[:])
                    ps_b = mmp.tile([128, 4], F32, tag="mm")
                    nc.tensor.matmul(out=ps_b[:], lhsT=ones_row1[:], rhs=sc16[:],
                                 start=True, stop=True)
                    musd = work.tile([128, 4], F32, tag="musd")
                    nc.vector.tensor_copy(out=musd[:], in_=ps_b[:])
                    _ctx["musd"] = musd
                    if not _last:
                        wm = work.tile([128, 1], F32, tag="wm")
                        nc.vector.tensor_tensor(
                            out=wm[:], in0=wbarT[:, _li:_li + 1],
                            in1=musd[:, 0:1], op=AOP.mult)
                        _ctx["wm"] = wm

                if last:
                    emit_stats_chain()
                else:
                    pending_chain = emit_stats_chain

            # ---------------- pool affine + MLP head ----------------
            if pool_ps is None:
                z = work.tile([128, NCLS], F32, tag="zz")
                nc.vector.memset(z[:], 0.0)
                nc.sync.dma_start(out=out_ext[0:128, :], in_=z[:])
                nc.sync.dma_start(out=out_ext[128:256, :], in_=z[:])
            else:
                pooledT = work.tile([128, NGRAPH], F32, tag="pooledT")
                nc.scalar.activation(out=pooledT[:], in_=pool_ps[:],
                                     func=AF.Copy, bias=0.0, scale=1.0)
                pl_in = dram2.tile([128, NGRAPH], F32, tag="plin")
                pl_out = dram2.tile([128, NGRAPH], F32, tag="plout")
                nc.sync.dma_start(out=pl_in[:], in_=pooledT[:])
                nc.gpsimd.collective_compute(
                    "AllReduce", AOP.add, replica_groups=rg,
                    ins=[pl_in[:]], outs=[pl_out[:]])
                pooled = work.tile([128, NGRAPH], F32, tag="pooled2")
                nc.sync.dma_start(out=pooled[:], in_=pl_out[:])
                nc.vector.tensor_tensor(out=pooled[:], in0=pooled[:],
                                        in1=invcntr[:], op=AOP.mult)
                mrs = work.tile([128, 1], F32, tag="mrs")
                nc.vector.tensor_tensor(out=mrs[:], in0=layer_ctx["musd"][:, 0:1],
                                        in1=layer_ctx["musd"][:, 3:4], op=AOP.mult)
                nc.vector.tensor_scalar(out=pooled[:], in0=pooled[:],
                                        scalar1=layer_ctx["musd"][:, 3:4],
                                        scalar2=mrs[:],
                                        op0=AOP.mult, op1=AOP.subtract)
                pooled16 = work.tile([128, NGRAPH], BF16, tag="pooled16")
                nc.vector.tensor_copy(out=pooled16[:], in_=pooled[:])

                ps_g = mmp.tile([128, NGRAPH], F32, tag="mm")
                nc.tensor.matmul(out=ps_g[:], lhsT=mlpW1[:], rhs=pooled16[:],
                                 start=True, stop=True)
                gT = work.tile([128, NGRAPH], BF16, tag="gT")
                nc.scalar.activation(out=gT[:], in_=ps_g[:], func=AF.Relu,
                                     bias=mlpb1[:], scale=1.0)
                for halfi in range(2):
                    ps_sc = mmp.tile([128, NCLS], F32, tag="mm")
                    nc.tensor.matmul(out=ps_sc[:],
                                     lhsT=gT[:, halfi * 128:(halfi + 1) * 128],
                                     rhs=mlpW2[:], start=True, stop=True)
                    scr = work.tile([128, NCLS], F32, tag="scr")
                    nc.vector.tensor_tensor(out=scr[:], in0=ps_sc[:],
                                            in1=mlpb2r[:], op=AOP.add)
                    mx = work.tile([128, 1], F32, tag="mx")
                    nc.vector.tensor_reduce(out=mx[:], in_=scr[:],
                                            axis=mybir.AxisListType.X,
                                            op=AOP.max)
                    nc.vector.tensor_scalar(out=scr[:], in0=scr[:], scalar1=mx[:],
                                            scalar2=None, op0=AOP.subtract)
                    ex = work.tile([128, NCLS], F32, tag="ex")
                    sm = work.tile([128, 1], F32, tag="sm")
                    nc.scalar.activation(out=ex[:], in_=scr[:], func=AF.Exp,
                                         bias=0.0, scale=1.0, accum_out=sm[:])
                    ls = work.tile([128, 1], F32, tag="ls")
                    nc.scalar.activation(out=ls[:], in_=sm[:], func=AF.Ln,
                                         bias=0.0, scale=1.0)
                    nc.vector.tensor_scalar(out=scr[:], in0=scr[:], scalar1=ls[:],
                                            scalar2=None, op0=AOP.subtract)
                    nc.sync.dma_start(
                        out=out_ext[halfi * 128:(halfi + 1) * 128, :],
                        in_=scr[:])

    nc.compile()
    return nc


def _wrap_cols(vec, fill):
    padded = np.full(NBLK * 128, fill, np.float32)
    padded[:NSH] = vec
    return np.ascontiguousarray(padded.reshape(NBLK, 128).T)


def _prepare(inputs):
    x = np.asarray(inputs["x"], dtype=np.float32)
    edge_index = np.asarray(inputs["edge_index"])
    batch = np.asarray(inputs["batch"], dtype=np.int64)

    dinv, sigdinv, idxw, slotw, meta = _host_preprocess(edge_index)

    cnt = np.bincount(batch, minlength=NGRAPH).astype(np.float64)
    invcnt = (1.0 / np.maximum(cnt, 1.0)).astype(np.float32)
    iota8 = np.tile(np.arange(128, dtype=np.float32), (128, TPC))
    iota256 = np.broadcast_to(np.arange(256, dtype=np.float32), (128, 256))

    lin1_W = np.asarray(inputs["lin1_W"], np.float32)
    lin1_b = np.asarray(inputs["lin1_b"], np.float32)
    conv_W = np.asarray(inputs["conv_W"], np.float32)
    conv_b = np.asarray(inputs["conv_b"], np.float32)
    mlp_W1 = np.asarray(inputs["mlp_W1"], np.float32)
    mlp_b1 = np.asarray(inputs["mlp_b1"], np.float32)
    mlp_W2 = np.asarray(inputs["mlp_W2"], np.float32)
    mlp_b2 = np.asarray(inputs["mlp_b2"], np.float32)

    convW_cat = np.concatenate([conv_W[l] for l in range(LAYERS)], axis=1)
    wbarT = np.stack([conv_W[l].sum(axis=0) for l in range(1, LAYERS)], axis=1)

    in_maps = []
    for c in range(NCORES):
        lo, hi = c * NSH, (c + 1) * NSH
        xT = np.zeros((F, NBLK * 128), np.float32)
        xT[:, :NSH] = x[lo:hi].T
        dinv_pad = np.zeros(NBLK * 128, np.float32)
        dinv_pad[:NSH] = dinv[lo:hi]
        sig_pad = np.zeros(NBLK * 128, np.float32)
        sig_pad[:NSH] = sigdinv[lo:hi]
        in_maps.append({
            "xT": xT.astype(BF),
            "idx": idxw[c],
            "slot": slotw[c],
            "dinvrep": np.ascontiguousarray(
                np.broadcast_to(dinv_pad, (128, NBLK * 128))).astype(BF),
            "sigdr": np.ascontiguousarray(
                np.broadcast_to(sig_pad, (128, NBLK * 128))).astype(BF),
            "dinvw": _wrap_cols(dinv[lo:hi], 0.0),
            "pslot": _wrap_cols(batch[lo:hi].astype(np.float32),
                                300.0).astype(BF),
            "iota8": iota8.astype(BF),
            "iota256": iota256.astype(BF),
            "lin1W": lin1_W.astype(BF),
            "lin1b": np.ascontiguousarray(lin1_b.reshape(F, 1)),
            "convW": convW_cat.astype(BF),
            "convb": np.ascontiguousarray(conv_b.T),
            "wbarT": np.ascontiguousarray(wbarT),
            "mlpW1": mlp_W1.astype(BF),
            "mlpb1": np.ascontiguousarray(mlp_b1.reshape(F, 1)),
            "mlpW2": mlp_W2.astype(BF),
            "mlpb2r": np.ascontiguousarray(
                np.broadcast_to(mlp_b2, (128, NCLS)).astype(np.float32)),
            "invcntr": np.ascontiguousarray(
                np.broadcast_to(invcnt, (128, NGRAPH))),
        })
    return meta, in_maps


_CACHED = {}


def kernel_run(inputs, trace=False):
    import os
    meta, in_maps = _prepare(inputs)
    key = (meta["TT"], os.environ.get("KCUT", "full"))
    if key not in _CACHED:
        _CACHED[key] = _build_program(meta)
    nc = _CACHED[key]
    res = run_bass_kernel_spmd(nc, in_maps, core_ids=list(range(NCORES)),
                               trace=trace)
    out = np.asarray(res.results[0]["out"], dtype=np.float32)
    return out, res.exec_time_ns


def kernel(**inputs):
    out, _ = kernel_run(inputs, trace=False)
    return out

